# revision 13
# baseline (speedup 1.0000x reference)
"""Trainium2 Bass kernel for nn_Encoder_45475113730366 (v4).

Data-parallel over batch (64 -> 8 cores x 8 items). v4 reworks v3 for
PE warmth + instruction-count:
  - Two-pass schedule: pass A runs m1/m2/e1/e2 (4 streams striped) for
    each item and stores enc0 = WL2-combine to DRAM scratch; pass B
    runs the enc chains of 4 items striped (2 groups) so the PE never
    sits idle on a single serial chain (HAM stays warm).
  - Column-centered Wq/W1 (host fold) absorb the LayerNorm mean
    subtraction into the weights: no cq/smr rank-1 corrections, no
    -mu*rstd row, broadcast halves to rstd-only.
  - LN stats in row form: ones-stationary matmuls (LDW P=1, stream
    bound) -> [8,512] PSUM -> one retile DMA -> [32,128] SBUF; rstd =
    Sqrt(reciprocal(var+eps)) (2 small ops, replaces quake rsqrt).
    LN1 transposes rstd back to token-partitioned form for the q-copy
    scale; LN2 keeps row form for the broadcast.
  - mish tail: z2 = Exp(h - ln2) (ACT), p' = (1+z2)*z2 fp16 (DVE STT),
    g = MISH_FINAL_ANT(p', h, s1=0.5) since p'/(p'+1/2) = p/(p+2).
    Saves the fp32 p pass of v3.
  - residuals stay on PE (identity matmuls); fp16 weights+activations.
"""
from contextlib import ExitStack

import numpy as np

import concourse.bacc as bacc
import concourse.bass as bass
import concourse.tile as tile
from concourse import mybir
from concourse import dve_ops
from concourse.dve_spec import (
    AluOp, Bin, Spec, Src0, Src1, C0, C1,
    _has_src1, lower as dve_lower,
)
from concourse.dve_uop import DveOpSpec
from concourse.masks import make_identity

N_CORES = 8
B, S, DM, H, DK, DI, L = 64, 2048, 128, 8, 16, 512, 4
DKP = DK // 2
HE = H * DKP          # 64 pooled kv features
NT = S // 128         # 16 token tiles
EPS = 1e-6
TEMP = 0.5 * float(np.sqrt(DK))
BIGB = 4096.0         # block-diag additive mask magnitude
RC0 = -0.23549792     # recip NOT-seed Chebyshev scale
RC1 = 2.0017324       # recip NOT-seed Chebyshev NR constant
LN2C = 0.6931471805599453

f32 = mybir.dt.float32
f16 = mybir.dt.float16
i32 = mybir.dt.int32
AX = mybir.AxisListType
OP = mybir.AluOpType
AF = mybir.ActivationFunctionType


def _register_mish_op():
    """out = in0 * recip_nr1(in0 + s1) * in1  (s0=seed scale, s1 runtime).

    Called with in0 = p' = (z2)(z2+1), z2 = e^h/2, s1 = 0.5, in1 = h:
    g = p'/(p'+0.5) * h = p/(p+2) * h = mish(h). 8 ALU slices."""
    for o in dve_ops.OPS:
        if o.name == "MISH_FINAL_ANT":
            return o
    from concourse.dve_spec import C2
    _den = Src0 + C1
    _nx = Bin(AluOp.BITWISE_NOT, _den, _den)
    _y0 = _nx * C0
    _y1 = _y0 * (C2 - _den * _y0)

    def _ref(in0, in1, c0, c1, c2):
        den = (np.asarray(in0, np.float32) + np.float32(c1))
        nx = (~den.view(np.int32)).view(np.float32)
        y0 = nx * np.float32(c0)
        y1 = y0 * (np.float32(c2) - den * y0)
        return (np.asarray(in0, np.float32) * y1) * np.asarray(in1, np.float32)

    spec = Spec(body=(Src0 * _y1) * Src1, reference=_ref)
    op = dve_ops.DveOp("MISH_FINAL_ANT", spec, subdim=False, uops_sha={})
    dve_ops.OPS.append(op)
    dve_ops.CUSTOM_DVE_SPECS[op.name] = op.spec
    dve_ops._SUB_OPCODE_FOR_NAME[op.name] = (
        dve_ops._CUSTOM_DVE_ROW_BASE + len(dve_ops.OPS) - 1)
    for ver in ("v3", "v4"):
        try:
            res = DveOpSpec(name=op.name,
                            opcode=dve_ops.get_dve_sub_opcode(op.name),
                            uops=dve_lower(op.spec, ver=ver),
                            rd1_en=_has_src1(op.spec))
            op.uops_sha[ver] = res.sha(ver)
        except Exception:
            pass
    return op


MISH_OP = _register_mish_op()


def fold_weights(inp):
    f = {}
    Wq = np.asarray(inp['Wq'], np.float32)
    Wk = np.asarray(inp['Wk'], np.float32)
    Wv = np.asarray(inp['Wv'], np.float32)
    Wfc = np.asarray(inp['Wfc'], np.float32)
    W1 = np.asarray(inp['W1'], np.float32)
    W2 = np.asarray(inp['W2'], np.float32)
    g1 = np.asarray(inp['ln1_g'], np.float32)
    b1n = np.asarray(inp['ln1_b'], np.float32)
    g2 = np.asarray(inp['ln2_g'], np.float32)
    b2n = np.asarray(inp['ln2_b'], np.float32)

    wq = (g1[:, :, None] * Wq) / TEMP                            # [L,128,128]
    f['wq'] = wq - wq.mean(axis=1, keepdims=True)                # centered
    f['bq'] = np.einsum('ld,ldf->lf', b1n, Wq) / TEMP
    f['wk'] = Wk.reshape(L, DM, H, DKP, 2).mean(-1).reshape(L, DM, HE)
    f['wv'] = Wv.reshape(L, DM, H, DKP, 2).mean(-1).reshape(L, DM, HE)
    perm = np.array([d * H + h for h in range(H) for d in range(DK)])
    f['wfc'] = Wfc[:, perm, :]                                   # [L,128,128]
    w1 = g2[:, :, None] * W1                                     # [L,128,512]
    f['w1'] = w1 - w1.mean(axis=1, keepdims=True)                # centered
    f['b1'] = np.einsum('ld,ldf->lf', b2n, W1) + np.asarray(inp['b1'], np.float32)
    f['w2r'] = W2.reshape(L, 4, 128, DM).transpose(0, 2, 1, 3).reshape(L, 128, 4 * DM)
    f['wl2'] = np.asarray(inp['WL2'], np.float32)                # [256,128]
    f['bl2'] = np.asarray(inp['bL2'], np.float32)[:, None]       # [128,1]
    f['b2s'] = np.ascontiguousarray(np.asarray(inp['b2'], np.float32).T)
    U8 = np.zeros((H, DM), np.float32)
    W8 = np.zeros((H, HE), np.float32)
    for h in range(H):
        U8[h, h * DK:(h + 1) * DK] = BIGB
        W8[h, h * DKP:(h + 1) * DKP] = 1.0
    f['u8'] = U8
    f['w8'] = W8
    mask = np.asarray(inp['src_mask'])
    f['maskbias'] = np.where(mask[:, :, None, :], 0.0, np.float32(-1e9)) \
        .astype(np.float32).repeat(H, axis=2).reshape(mask.shape[0], DK, HE)
    f['maskbias_full'] = np.tile(f['maskbias'], (1, H, 1))       # [B,128,HE]
    f['mask_trivial'] = bool(mask.all())
    f['bq_trivial'] = bool(np.abs(f['bq']).max() == 0.0)
    f['b1_trivial'] = bool(np.abs(f['b1']).max() == 0.0)
    return f


def build(n_items, use_mask):
    assert n_items % 4 == 0
    ng = n_items // 4
    nc = bacc.Bacc(trn_type="TRN2", target_bir_lowering=False, debug=False)

    # ---- DRAM tensors -------------------------------------------------
    xin = nc.dram_tensor("xin", [n_items, 2 * DM, S], f16, kind="ExternalInput").ap()
    wq_d = nc.dram_tensor("wq", [L, DM, DM], f16, kind="ExternalInput").ap()
    wk_d = nc.dram_tensor("wk", [L, DM, HE], f16, kind="ExternalInput").ap()
    wv_d = nc.dram_tensor("wv", [L, DM, HE], f16, kind="ExternalInput").ap()
    wfc_d = nc.dram_tensor("wfc", [L, DM, DM], f16, kind="ExternalInput").ap()
    w1_d = nc.dram_tensor("w1", [L, DM, DI], f16, kind="ExternalInput").ap()
    w2_d = nc.dram_tensor("w2r", [L, DM, DI], f16, kind="ExternalInput").ap()
    u8_d = nc.dram_tensor("u8", [H, DM], f16, kind="ExternalInput").ap()
    w8_d = nc.dram_tensor("w8", [H, HE], f16, kind="ExternalInput").ap()
    b2_d = nc.dram_tensor("b2s", [DM, L], f32, kind="ExternalInput").ap()
    wl2_d = nc.dram_tensor("wl2", [2 * DM, DM], f16, kind="ExternalInput").ap()
    bl2_d = nc.dram_tensor("bl2", [DM, 1], f32, kind="ExternalInput").ap()
    if use_mask:
        mb_d = nc.dram_tensor("mb", [n_items, DM, HE], f32, kind="ExternalInput").ap()
    m1_o = nc.dram_tensor("m1o", [n_items, DM, S], f16, kind="ExternalOutput").ap()
    m2_o = nc.dram_tensor("m2o", [n_items, DM, S], f16, kind="ExternalOutput").ap()
    e_o = nc.dram_tensor("eo", [ng, 4, DM, S], f16, kind="ExternalOutput").ap()
    # DRAM scratch for the enc0 states between pass A and pass B
    enc_s = nc.dram_tensor("encs", [ng, 4, DM, S], f16, kind="ExternalOutput").ap()

    with tile.TileContext(nc) as tc, ExitStack() as ctx:
        consts = ctx.enter_context(tc.tile_pool(name="consts", bufs=1))
        xp = ctx.enter_context(tc.tile_pool(name="xp", bufs=2))
        statep = ctx.enter_context(tc.tile_pool(name="statep", bufs=6))
        qp = ctx.enter_context(tc.tile_pool(name="qp", bufs=4))
        kp = ctx.enter_context(tc.tile_pool(name="kp", bufs=4))
        vp = ctx.enter_context(tc.tile_pool(name="vp", bufs=4))
        outp = ctx.enter_context(tc.tile_pool(name="outp", bufs=4))
        sqp = ctx.enter_context(tc.tile_pool(name="sqp", bufs=3))
        zp = ctx.enter_context(tc.tile_pool(name="zp", bufs=3))
        pp = ctx.enter_context(tc.tile_pool(name="pp", bufs=3))
        gp = ctx.enter_context(tc.tile_pool(name="gp", bufs=6))
        n2p = ctx.enter_context(tc.tile_pool(name="n2p", bufs=6))
        tinyp = ctx.enter_context(tc.tile_pool(name="tinyp", bufs=8))
        rowp = ctx.enter_context(tc.tile_pool(name="rowp", bufs=3))
        ps_mm = ctx.enter_context(tc.tile_pool(name="ps_mm", bufs=2, space="PSUM"))
        ps_ty = ctx.enter_context(tc.tile_pool(name="ps_ty", bufs=2, space="PSUM"))
        ps_h = ctx.enter_context(tc.tile_pool(name="ps_h", bufs=2, space="PSUM"))

        # ---- constants / weights into SBUF ---------------------------
        identf = consts.tile([128, 128], f32, tag="identf")
        make_identity(nc, identf)
        ident = consts.tile([128, 128], f16, tag="ident")
        nc.vector.tensor_copy(ident, identf)
        ones128 = consts.tile([128, 1], f16, tag="ones128")
        nc.vector.memset(ones128, 1.0 / 128.0)
        nln2x4 = consts.tile([128, 1], f32, tag="nln2x4")
        nc.vector.memset(nln2x4, -4.0 * LN2C)

        def _load(name, dram_ap, shape, dt=f16):
            t = consts.tile(list(shape), dt, tag=name)
            nc.sync.dma_start(out=t, in_=dram_ap)
            return t

        wq_sb = [_load(f"wq{i}", wq_d[i], [128, DM]) for i in range(L)]
        wk_sb = [_load(f"wk{i}", wk_d[i], [128, HE]) for i in range(L)]
        wv_sb = [_load(f"wv{i}", wv_d[i], [128, HE]) for i in range(L)]
        wfc_sb = [_load(f"wfc{i}", wfc_d[i], [128, DM]) for i in range(L)]
        w1_sb = [_load(f"w1{i}", w1_d[i], [128, DI]) for i in range(L)]
        w2_sb = [_load(f"w2{i}", w2_d[i], [128, DI]) for i in range(L)]
        u8_sb = _load("u8", u8_d, [H, DM])
        w8_sb = _load("w8", w8_d, [H, HE])
        b2_sb = _load("b2s", b2_d, [128, L], f32)
        wl2a = _load("wl2a", wl2_d[0:DM], [128, DM])
        wl2b = _load("wl2b", wl2_d[DM:2 * DM], [128, DM])
        bl2_sb = _load("bl2", bl2_d, [128, 1], f32)

        def ln_stats(x, sq_on_pool):
            """x: [128,2048] f16 fm. Returns st_ps [128,32] (mu | e2)."""
            st_ps = ps_ty.tile([128, 32], f32, tag="typs")
            for t in range(NT):
                nc.tensor.matmul(st_ps[:, t:t + 1],
                                 lhsT=x[:, t * 128:(t + 1) * 128], rhs=ones128)
            for c in range(4):
                sqc = sqp.tile([128, 512], f16, tag="sqc")
                eng = nc.gpsimd if sq_on_pool else nc.vector
                eng.tensor_mul(sqc, x[:, c * 512:(c + 1) * 512],
                               x[:, c * 512:(c + 1) * 512])
                for tt in range(4):
                    t = 4 * c + tt
                    nc.tensor.matmul(st_ps[:, 16 + t:17 + t],
                                     lhsT=sqc[:, tt * 128:(tt + 1) * 128],
                                     rhs=ones128)
            return st_ps

        QK = 0x5f3759df
        def rsqrt_pos(v, pfx):
            """quake rsqrt, 2 newton iters -> +rstd (fp32 [128,16])."""
            yi = tinyp.tile([128, 16], i32, tag=pfx + "yi")
            nc.vector.tensor_scalar(out=yi, in0=v.bitcast(i32), scalar1=1,
                                    scalar2=None, op0=OP.arith_shift_right)
            nc.vector.tensor_scalar(out=yi, in0=yi, scalar1=-1,
                                    scalar2=None, op0=OP.bitwise_xor)
            nc.vector.tensor_scalar(out=yi, in0=yi, scalar1=QK + 1,
                                    scalar2=None, op0=OP.add)
            y = yi.bitcast(f32)
            hv = tinyp.tile([128, 16], f32, tag=pfx + "hv")
            nc.vector.tensor_scalar(out=hv, in0=v, scalar1=0.5, scalar2=None,
                                    op0=OP.mult)
            tq = tinyp.tile([128, 16], f32, tag=pfx + "tq")
            for _ in range(2):
                nc.vector.tensor_mul(tq, y, y)
                nc.vector.tensor_mul(tq, tq, hv)
                nc.vector.scalar_tensor_tensor(out=y, in0=tq, scalar=1.5, in1=y,
                                               op0=OP.subtract, op1=OP.mult)
            return y

        def ln_rstd(st_ps, pfx):
            """rstd = quake_rsqrt(var+eps): pure DVE, no ACT table switch."""
            mu = st_ps[:, 0:16]
            e2 = st_ps[:, 16:32]
            musq = tinyp.tile([128, 16], f32, tag=pfx + "musq")
            nc.scalar.activation(musq, mu, AF.Square)
            vpe = tinyp.tile([128, 16], f32, tag=pfx + "vpe")
            nc.vector.scalar_tensor_tensor(out=vpe, in0=e2, scalar=float(EPS),
                                           in1=musq, op0=OP.add, op1=OP.subtract)
            return rsqrt_pos(vpe, pfx)

        def ph1_stats(st):
            st_ps = ln_stats(st['x'], sq_on_pool=True)
            st['rstd'] = ln_rstd(st_ps, "a")

        def ph2_qkv(st, i):
            xq, xkv, rstd = st['x'], st['kv'], st['rstd']
            q_sb = qp.tile([128, S], f16, tag="q")
            for b4 in range(4):
                qr_ps = ps_mm.tile([128, 512], f32, tag="mm")
                for tt in range(4):
                    t = 4 * b4 + tt
                    nc.tensor.matmul(qr_ps[:, tt * 128:(tt + 1) * 128],
                                     lhsT=xq[:, t * 128:(t + 1) * 128],
                                     rhs=wq_sb[i])
                for tt in range(4):
                    t = 4 * b4 + tt
                    nc.scalar.activation(q_sb[:, t * 128:(t + 1) * 128],
                                         qr_ps[:, tt * 128:(tt + 1) * 128],
                                         AF.Copy, scale=rstd[:, t:t + 1])
            k_sb = kp.tile([128, NT * HE], f16, tag="k")
            for b2 in range(2):
                k_ps = ps_mm.tile([128, 512], f32, tag="mm")
                for tt in range(8):
                    t = 8 * b2 + tt
                    nc.tensor.matmul(k_ps[:, tt * HE:(tt + 1) * HE],
                                     lhsT=xkv[:, t * 128:(t + 1) * 128],
                                     rhs=wk_sb[i])
                nc.vector.tensor_copy(k_sb[:, b2 * 512:(b2 + 1) * 512], k_ps)
            vT = vp.tile([HE, S], f16, tag="vT")
            for c in range(4):
                v_ps = ps_mm.tile([HE, 512], f32, tag="mm")
                nc.tensor.matmul(v_ps, lhsT=wv_sb[i],
                                 rhs=xkv[:, c * 512:(c + 1) * 512])
                nc.scalar.copy(out=vT[:, c * 512:(c + 1) * 512], in_=v_ps)
            st['q'], st['k'], st['vT'] = q_sb, k_sb, vT

        def ph3_attn(st, i, mb_sb):
            q_sb, k_sb = st['q'], st['k']
            s_ps = ps_ty.tile([128, HE], f32, tag="typs")
            for t in range(NT):
                nc.tensor.matmul(s_ps, lhsT=q_sb[:, t * 128:(t + 1) * 128],
                                 rhs=k_sb[:, t * HE:(t + 1) * HE],
                                 start=(t == 0), stop=False)
            nc.tensor.matmul(s_ps, lhsT=u8_sb, rhs=w8_sb,
                             start=False, stop=True)
            if use_mask:
                nc.vector.tensor_add(s_ps, s_ps, mb_sb)
            nmx = tinyp.tile([128, 1], f32, tag="nmx")
            nc.vector.tensor_reduce(nmx, s_ps, axis=AX.X, op=OP.max, negate=True)
            es = tinyp.tile([128, HE], f16, tag="es")
            asum = tinyp.tile([128, 1], f32, tag="asum")
            nc.scalar.activation(es, s_ps, AF.Exp, bias=nmx, accum_out=asum)
            rs = tinyp.tile([128, 1], f32, tag="rs")
            nc.vector.reciprocal(rs, asum)
            bda = tinyp.tile([128, HE], f16, tag="bda")
            nc.vector.tensor_scalar(out=bda, in0=es, scalar1=rs, scalar2=None,
                                    op0=OP.mult)
            c_ps = ps_ty.tile([HE, 128], f32, tag="typs")
            nc.tensor.matmul(c_ps, lhsT=bda, rhs=wfc_sb[i])
            c_sb = tinyp.tile([HE, 128], f16, tag="csb")
            nc.vector.tensor_copy(c_sb, c_ps)
            st['c'] = c_sb

        def ph4_out1(st):
            xq, vT, c_sb = st['x'], st['vT'], st['c']
            out1 = outp.tile([128, S], f16, tag="out1")
            for c in range(4):
                cs = slice(c * 512, (c + 1) * 512)
                ofc_ps = ps_mm.tile([128, 512], f32, tag="mm")
                nc.tensor.matmul(ofc_ps, lhsT=c_sb, rhs=vT[:, cs],
                                 start=True, stop=False)
                nc.tensor.matmul(ofc_ps, lhsT=ident, rhs=xq[:, cs],
                                 start=False, stop=True)
                nc.scalar.copy(out=out1[:, cs], in_=ofc_ps)
            st['out1'] = out1

        def ph5_ln2(st):
            out1 = st['out1']
            st_ps = ln_stats(out1, sq_on_pool=False)
            rstd2 = ln_rstd(st_ps, "b")
            tr_ps = ps_ty.tile([16, 128], f32, tag="typs")
            nc.tensor.transpose(tr_ps, rstd2, identf)
            rs16h = rowp.tile([16, 128], f16, tag="rs16h")
            nc.vector.tensor_copy(rs16h, tr_ps)
            rowrow = rowp.tile([1, S], f16, tag="rowrow")
            nc.sync.dma_start(out=rowrow, in_=rs16h)
            rb = rowp.tile([128, S], f16, tag="rb")
            nc.gpsimd.partition_broadcast(rb, rowrow)
            n2cs = []
            for c2 in range(2):
                cs = slice(c2 * 1024, (c2 + 1) * 1024)
                n2c = n2p.tile([128, 1024], f16, tag="n2c")
                nc.vector.tensor_mul(n2c, out1[:, cs], rb[:, cs])
                n2cs.append(n2c)
            st['n2cs'] = n2cs

        def ph6_ffn(st, i):
            out1 = st['out1']
            out2 = statep.tile([128, S], f16, tag="state")
            for c2 in range(2):
                n2c = st['n2cs'][c2]
                gs = []
                for j in range(4):
                    h_ps = ps_h.tile([128, 1024], f32, tag="hps")
                    for hh in range(2):
                        hsl = slice(hh * 512, (hh + 1) * 512)
                        nc.tensor.matmul(h_ps[:, hsl],
                                         lhsT=w1_sb[i][:, j * 128:(j + 1) * 128],
                                         rhs=n2c[:, hsl],
                                         start=True, stop=True)
                    # z = e^h/16; p' = z^2 + z/8 = p/256 (fp16-safe to h~8.3)
                    # g = p'/(p' + 2/256) * h = mish(h)
                    z = zp.tile([128, 1024], f16, tag="z")
                    nc.scalar.activation(z, h_ps, AF.Exp, bias=nln2x4)
                    p = pp.tile([128, 1024], f16, tag="p")
                    nc.vector.scalar_tensor_tensor(out=p, in0=z, scalar=0.125,
                                                   in1=z, op0=OP.add, op1=OP.mult)
                    g = gp.tile([128, 1024], f16, tag="hsb")
                    nc.vector._custom_dve(MISH_OP, out=g, in0=p, in1=h_ps,
                                          s0=RC0, s1=2.0 / 256.0, imm2=RC1)
                    gs.append(g)
                for hf in range(2):
                    hs = slice(hf * 512, (hf + 1) * 512)
                    cso = slice(c2 * 1024 + hf * 512, c2 * 1024 + (hf + 1) * 512)
                    o_ps = ps_mm.tile([128, 512], f32, tag="mm")
                    for j in range(4):
                        nc.tensor.matmul(o_ps,
                                         lhsT=w2_sb[i][:, j * 128:(j + 1) * 128],
                                         rhs=gs[j][:, hs],
                                         start=(j == 0), stop=False)
                    nc.tensor.matmul(o_ps, lhsT=ident, rhs=out1[:, cso],
                                     start=False, stop=True)
                    nc.scalar.activation(out2[:, cso], o_ps, AF.Identity,
                                         bias=b2_sb[:, i:i + 1])
            return out2

        def emit_layers(streams, i, mb_sbs):
            for st in streams:
                ph1_stats(st)
            for st in streams:
                ph2_qkv(st, i)
            for st, mb in zip(streams, mb_sbs):
                ph3_attn(st, i, mb)
            for st in streams:
                ph4_out1(st)
            for st in streams:
                ph5_ln2(st)
            outs = []
            for st in streams:
                outs.append(ph6_ffn(st, i))
            for st, o in zip(streams, outs):
                st['x'] = o

        # ---- pass A: m1/m2/e1/e2 chains per item, store enc0 ---------
        with tc.For_i(0, n_items, 1, staggered_reset=True) as it:
            xitem = xin[bass.ds(it, 1)].squeeze(0)     # [256 feat, S] f16 DRAM
            x1 = xp.tile([128, S], f16, tag="x1")
            x2 = xp.tile([128, S], f16, tag="x2")
            nc.sync.dma_start(out=x1, in_=xitem[0:DM])
            nc.sync.dma_start(out=x2, in_=xitem[DM:2 * DM])
            if use_mask:
                mb_sb = tinyp.tile([DM, HE], f32, tag="mb")
                nc.sync.dma_start(out=mb_sb, in_=mb_d[bass.ds(it, 1)].squeeze(0))
            else:
                mb_sb = None
            mb_sbs = [mb_sb] * 4

            sts = [{'x': x1}, {'x': x2}, {'x': x2}, {'x': x1}]
            for i in range(L):
                sts[0]['kv'] = sts[0]['x']
                sts[1]['kv'] = sts[1]['x']
                sts[2]['kv'] = x1 if i == 0 else sts[2]['x']
                sts[3]['kv'] = x2 if i == 0 else sts[3]['x']
                emit_layers(sts, i, mb_sbs)
            nc.sync.dma_start(out=m1_o[bass.ds(it, 1)].squeeze(0), in_=sts[0]['x'])
            nc.sync.dma_start(out=m2_o[bass.ds(it, 1)].squeeze(0), in_=sts[1]['x'])

            # enc0 = concat(e1,e2) @ WL2 + bL2 -> DRAM scratch
            eA, eB = sts[2]['x'], sts[3]['x']
            enc = statep.tile([128, S], f16, tag="state")
            for c2 in range(2):
                cs = slice(c2 * 1024, (c2 + 1) * 1024)
                en_ps = ps_h.tile([128, 1024], f32, tag="hps")
                for hh in range(2):
                    hsl = slice(hh * 512, (hh + 1) * 512)
                    csl = slice(c2 * 1024 + hh * 512, c2 * 1024 + (hh + 1) * 512)
                    nc.tensor.matmul(en_ps[:, hsl], lhsT=wl2a, rhs=eA[:, csl],
                                     start=True, stop=False)
                    nc.tensor.matmul(en_ps[:, hsl], lhsT=wl2b, rhs=eB[:, csl],
                                     start=False, stop=True)
                nc.vector.tensor_scalar(out=enc[:, cs], in0=en_ps,
                                        scalar1=bl2_sb, scalar2=None, op0=OP.add)
            es_g = enc_s.rearrange("g j d s -> (g j) d s")
            nc.sync.dma_start(out=es_g[bass.ds(it, 1)].squeeze(0), in_=enc)

        # ---- pass B: enc chains, 4 items striped per group -----------
        with tc.For_i(0, ng, 1, staggered_reset=True) as g:
            encg = enc_s[bass.ds(g, 1)].squeeze(0)     # [4, DM, S]
            ests = []
            for j in range(4):
                xe = statep.tile([128, S], f16, tag="state")
                nc.sync.dma_start(out=xe, in_=encg[j])
                ests.append({'x': xe})
            if use_mask:
                mbs = []
                mbg = mb_d.rearrange("(g j) d e -> g j d e", j=4)[bass.ds(g, 1)].squeeze(0)
                for j in range(4):
                    mb_sb = tinyp.tile([DM, HE], f32, tag="mb")
                    nc.sync.dma_start(out=mb_sb, in_=mbg[j])
                    mbs.append(mb_sb)
            else:
                mbs = [None] * 4
            for i in range(L):
                for st in ests:
                    st['kv'] = st['x']
                emit_layers(ests, i, mbs)
            eog = e_o[bass.ds(g, 1)].squeeze(0)
            for j in range(4):
                nc.sync.dma_start(out=eog[j], in_=ests[j]['x'])

    nc.compile()
    return nc


_CACHE = {}


def _get_built(n_items, use_mask):
    key = (n_items, use_mask)
    if key not in _CACHE:
        _CACHE[key] = build(n_items, use_mask)
    return _CACHE[key]


def _in_maps(f, src_f16, n_items, n_cores, use_mask):
    tof = lambda a: np.ascontiguousarray(np.asarray(a, np.float32).astype(np.float16))
    base = {
        'wq': tof(f['wq']), 'wk': tof(f['wk']), 'wv': tof(f['wv']),
        'wfc': tof(f['wfc']), 'w1': tof(f['w1']), 'w2r': tof(f['w2r']),
        'u8': tof(f['u8']), 'w8': tof(f['w8']),
        'b2s': np.ascontiguousarray(f['b2s'], dtype=np.float32),
        'wl2': tof(f['wl2']),
        'bl2': np.ascontiguousarray(f['bl2'], dtype=np.float32),
    }
    maps = []
    for c in range(n_cores):
        m = dict(base)
        m['xin'] = src_f16[c * n_items:(c + 1) * n_items]
        if use_mask:
            m['mb'] = np.ascontiguousarray(
                f['maskbias_full'][c * n_items:(c + 1) * n_items], np.float32)
        maps.append(m)
    return maps


def run(inputs, trace=False):
    from concourse import bass_utils
    from concourse.bass_utils import run_bass_kernel_spmd
    if trace:
        import ntff_shim
        ntff_shim.install()
        bass_utils.upload_artifacts = lambda tmpdir: tmpdir
    f = fold_weights(inputs)
    use_mask = not f['mask_trivial']
    assert f['bq_trivial'] and f['b1_trivial'], \
        "v4 kernel folds LN means into centered weights; needs zero ln biases"
    src = np.asarray(inputs['src_seq'], np.float32)
    nb = src.shape[0]
    n_cores = N_CORES if nb % N_CORES == 0 else 1
    n_items = nb // n_cores
    src_f16 = np.ascontiguousarray(src.transpose(0, 2, 1).astype(np.float16))
    nc = _get_built(n_items, use_mask)
    maps = _in_maps(f, src_f16, n_items, n_cores, use_mask)
    res = run_bass_kernel_spmd(nc, maps, core_ids=list(range(n_cores)),
                               trace=trace, trace_cores=[0] if trace else None)

    def gather(name):
        a = np.concatenate([np.asarray(res.results[c][name]).reshape(-1, DM, S)
                            for c in range(n_cores)], 0)
        return np.ascontiguousarray(
            a.astype(np.float32).transpose(0, 2, 1))

    return (gather('eo'), gather('m1o'), gather('m2o')), res


def kernel(**inputs):
    (enc, m1, m2), _ = run(inputs, trace=False)
    return (enc, m1, m2)
